# revision 1
# baseline (speedup 1.0000x reference)
"""MLA decode kernel for 8 TRN2 NeuronCores.

Sharding: batch-parallel — core b handles batch element b (B=8, n_cores=8).
Each core runs the full projection chain (weights replicated, bf16,
host-pre-transposed into PE-friendly layouts) plus attention over its own
batch's KV cache. The KV cache is fed in BOTH layouts ([c,t] for the score
matmul and [t,c] for the value matmul) as bf16, so no on-chip transposes of
the big cache are needed and total cache DMA bytes equal one f32 copy.

All matmuls run in bf16 with f32 PSUM accumulation. Softmax skips the
max-subtraction (scores are O(3), exp is safe in f32) and normalizes after
the value matmul.
"""
import numpy as np
import ml_dtypes

import concourse.bacc as bacc
import concourse.mybir as mybir
from concourse import bass_utils
from concourse.tile import TileContext
from concourse.masks import make_identity

BF = mybir.dt.bfloat16
F32 = mybir.dt.float32
npbf = ml_dtypes.bfloat16

N_CORES = 8
B, S, DIM = 8, 1, 2048
H = 16
QLR, KVLR = 1536, 512
DN, DR, DV = 128, 64, 128
TP = 8191            # prefix length
T = TP + 1           # 8192 total positions
SCALE = float((DN + DR) ** -0.5)
EPS = 1e-6
TBW = 1024           # t-block width
NTB = T // TBW       # 4 blocks

_NC_CACHE = {}


def _chunked(ap_dram, p=128):
    # [K, M] dram AP -> [p, K//p, M] iteration view (partition-major)
    return ap_dram.rearrange("(n p) m -> p n m", p=p)


def _build():
    if "nc" in _NC_CACHE:
        return _NC_CACHE["nc"]
    nc = bacc.Bacc("TRN2", target_bir_lowering=False, debug=False,
                   num_devices=N_CORES)
    I = {}

    def inp(name, shape, dt=BF):
        I[name] = nc.dram_tensor(name, shape, dt, kind="ExternalInput")
        return I[name]

    inp("xT16", [128, 16])
    inp("kvT", [KVLR, TP])
    inp("kvn", [TP, KVLR])
    inp("peT", [DR, TP])
    inp("wqaT", [DIM, QLR])
    inp("wqbT", [QLR, H * (DN + DR)])
    inp("wkvaT", [DIM, KVLR + DR])
    inp("wbk", [H * DN, KVLR])        # (h,d) x c
    inp("wbv", [4 * 128, H * DV])     # (cc,c) x (h,d)
    inp("woT", [H * DV, DIM])
    inp("qnw", [1, QLR], F32)
    inp("kvnw", [1, KVLR], F32)
    inp("wqab", [1, QLR], F32)
    inp("wqbb", [1, H * (DN + DR)], F32)
    inp("wkvab", [1, KVLR + DR], F32)
    inp("wob", [1, DIM], F32)
    inp("cosq", [1, H * 32], F32)
    inp("sinq", [1, H * 32], F32)
    out_d = nc.dram_tensor("out", [1, DIM], F32, kind="ExternalOutput")

    with TileContext(nc) as tc:
        _program(nc, tc, I, out_d)
    nc.compile()
    _NC_CACHE["nc"] = nc
    return nc


def _program(nc, tc, I, out_d):
    AL = mybir.AluOpType
    AF = mybir.ActivationFunctionType

    with (
        tc.tile_pool(name="consts", bufs=1) as cp,
        tc.tile_pool(name="wstream", bufs=2) as wp,
        tc.tile_pool(name="wconst", bufs=1) as wc,
        tc.tile_pool(name="kvTp", bufs=3) as kvTp,
        tc.tile_pool(name="kvnp", bufs=2) as kvnp,
        tc.tile_pool(name="pep", bufs=2) as pep,
        tc.tile_pool(name="attn", bufs=3) as atp,
        tc.tile_pool(name="ps_scores", bufs=2, space="PSUM") as pps,
        tc.tile_pool(name="ps_acc", bufs=1, space="PSUM") as ppa,
        tc.tile_pool(name="ps_tr", bufs=2, space="PSUM") as ppt,
        tc.tile_pool(name="ps_stage", bufs=1, space="PSUM") as ppg,
    ):
        # identities for PE transpose
        id_bf = cp.tile([128, 128], BF)
        id_f = cp.tile([128, 128], F32)
        make_identity(nc, id_bf[:])
        make_identity(nc, id_f[:])

        def load_const(name, dt=F32):
            t = cp.tile(list(I[name].shape), dt, tag=name)
            nc.sync.dma_start(out=t[:], in_=I[name].ap())
            return t

        xT = load_const("xT16", BF)
        qnw = load_const("qnw")
        kvnw = load_const("kvnw")
        wqab = load_const("wqab")
        wqbb = load_const("wqbb")
        wkvab = load_const("wkvab")
        wob = load_const("wob")
        cosq = load_const("cosq")
        sinq = load_const("sinq")

        # ---- GEMV helper: y[1, M] f32 = xT_cols.T @ w  (+ bias) ----
        def gemv(xT_sb, nk, w_name, M, bias_sb, out_sb):
            wd = I[w_name].ap()
            for mb0 in range(0, M, 512):
                mw = min(512, M - mb0)
                wt = wp.tile([128, 16, 512], BF, tag="wstream")
                nc.sync.dma_start(
                    out=wt[:, :nk, :mw],
                    in_=_chunked(wd)[:, :, mb0:mb0 + mw],
                )
                ps = ppg.tile([1, 512], F32, tag="stage")
                for kc in range(nk):
                    nc.tensor.matmul(
                        ps[:, :mw], xT_sb[:, kc:kc + 1], wt[:, kc, :mw],
                        start=(kc == 0), stop=(kc == nk - 1),
                    )
                nc.vector.tensor_tensor(
                    out=out_sb[:, mb0:mb0 + mw], in0=ps[:, :mw],
                    in1=bias_sb[:, mb0:mb0 + mw], op=AL.add,
                )

        # ---- rms helper: out_f32[1,N] = in[1,N]*w*rsqrt(mean(in^2)+eps) ----
        def rmsnorm(in_view, N, w_sb, w_off, out_sb):
            sq = cp.tile([1, 1536], F32, tag="scratch")
            ssq = cp.tile([1, 1], F32, tag=f"ssq{N}_{w_off}")
            nc.scalar.activation(out=sq[:, :N], in_=in_view, func=AF.Square,
                                 accum_out=ssq[:])
            ms = cp.tile([1, 1], F32, tag=f"ms{N}_{w_off}")
            nc.vector.tensor_scalar(out=ms[:], in0=ssq[:], scalar1=1.0 / N,
                                    scalar2=EPS, op0=AL.mult, op1=AL.add)
            sd = cp.tile([1, 1], F32, tag=f"sd{N}_{w_off}")
            nc.scalar.activation(out=sd[:], in_=ms[:], func=AF.Sqrt)
            rstd = cp.tile([1, 1], F32, tag=f"rstd{N}_{w_off}")
            nc.vector.reciprocal(out=rstd[:], in_=sd[:])
            tmp = cp.tile([1, 1536], F32, tag="scratch")
            nc.vector.tensor_tensor(out=tmp[:, :N], in0=in_view,
                                    in1=w_sb[:, :N], op=AL.mult)
            nc.vector.tensor_scalar(out=out_sb, in0=tmp[:, :N],
                                    scalar1=rstd[:], scalar2=None, op0=AL.mult)

        # ---- transpose helper: [1, n] f32 row -> psum col [n, 1] ----
        def trans_row(in_view, n, ps_out):
            nc.tensor.transpose(ps_out, in_view, id_f[0:1, 0:1])

        # ================= Q branch =================
        qa = cp.tile([1, QLR], F32)
        gemv(xT, 16, "wqaT", QLR, wqab, qa)
        qan = cp.tile([1, QLR], F32)
        rmsnorm(qa[:], QLR, qnw, 0, qan[:])

        # transpose q_a_n -> [128, 12] bf16
        pt_qa = ppt.tile([128, 64], F32, tag="tr")
        for kc in range(12):
            trans_row(qan[:, kc * 128:(kc + 1) * 128], 128,
                      pt_qa[:, kc:kc + 1])
        qaT = cp.tile([128, 12], BF)
        nc.scalar.copy(out=qaT[:], in_=pt_qa[:, :12])

        q = cp.tile([1, H * (DN + DR)], F32)
        gemv(qaT, 12, "wqbT", H * (DN + DR), wqbb, q)

        # rope(q_pe): strided views over all 16 heads at once
        qv = q[:].rearrange("b (h r) -> b h r", h=H)
        xr = qv[:, :, 128:192:2]
        xi = qv[:, :, 129:192:2]
        cosv = cosq[:].rearrange("b (h j) -> b h j", h=H)
        sinv = sinq[:].rearrange("b (h j) -> b h j", h=H)
        rp = cp.tile([1, H * DR], F32)
        rpv = rp[:].rearrange("b (h r) -> b h r", h=H)
        s1 = cp.tile([1, 512], F32, tag="rs1")
        s2 = cp.tile([1, 512], F32, tag="rs2")
        s1v = s1[:].rearrange("b (h j) -> b h j", h=H)
        s2v = s2[:].rearrange("b (h j) -> b h j", h=H)
        nc.vector.tensor_tensor(out=s1v, in0=xi, in1=sinv, op=AL.mult)
        nc.vector.tensor_tensor(out=s2v, in0=xr, in1=cosv, op=AL.mult)
        nc.vector.tensor_tensor(out=rpv[:, :, 0:64:2], in0=s2v, in1=s1v,
                                op=AL.subtract)
        nc.vector.tensor_tensor(out=s1v, in0=xr, in1=sinv, op=AL.mult)
        nc.vector.tensor_tensor(out=s2v, in0=xi, in1=cosv, op=AL.mult)
        nc.vector.tensor_tensor(out=rpv[:, :, 1:64:2], in0=s1v, in1=s2v,
                                op=AL.add)

        # q_nope -> [128, 16] bf16 (transposed)
        pt_qn = ppt.tile([128, 64], F32, tag="tr")
        for h in range(H):
            trans_row(q[:, h * 192:h * 192 + 128], 128, pt_qn[:, h:h + 1])
        qnT = cp.tile([128, H], BF)
        nc.scalar.copy(out=qnT[:], in_=pt_qn[:, :H])

        # absorption: qT[cc][128c, 16h] bf16
        wbk_sb = wc.tile([128, H, KVLR], BF)
        nc.sync.dma_start(out=wbk_sb[:], in_=_chunked(I["wbk"].ap()))
        qT = []
        for cc in range(4):
            ps_ab = ppt.tile([128, 64], F32, tag="tr")
            for h in range(H):
                nc.tensor.matmul(
                    ps_ab[:, h:h + 1],
                    wbk_sb[:, h, cc * 128:(cc + 1) * 128],
                    qnT[:, h:h + 1], start=True, stop=True,
                )
            t = cp.tile([128, H], BF, tag=f"qT{cc}")
            nc.scalar.copy(out=t[:], in_=ps_ab[:, :H])
            qT.append(t)

        # q_pe transposed -> [64, 16] bf16
        pt_qp = ppt.tile([128, 64], F32, tag="tr")
        for h in range(H):
            trans_row(rp[:, h * 64:(h + 1) * 64], 64, pt_qp[:64, h:h + 1])
        qpT = cp.tile([64, H], BF)
        nc.scalar.copy(out=qpT[:], in_=pt_qp[:64, :H])

        # ================= KV branch =================
        kvpe = cp.tile([1, KVLR + DR], F32)
        gemv(xT, 16, "wkvaT", KVLR + DR, wkvab, kvpe)
        kvn_f = cp.tile([1, KVLR], F32)
        rmsnorm(kvpe[:, :KVLR], KVLR, kvnw, 1, kvn_f[:])

        # rope(k_pe) -> kpe [1, 64] f32
        kpe = cp.tile([1, DR], F32)
        kxr = kvpe[:, KVLR + 0:KVLR + 64:2]
        kxi = kvpe[:, KVLR + 1:KVLR + 64:2]
        ks1 = cp.tile([1, 32], F32, tag="krs1")
        ks2 = cp.tile([1, 32], F32, tag="krs2")
        nc.vector.tensor_tensor(out=ks1[:], in0=kxi, in1=sinq[:, :32], op=AL.mult)
        nc.vector.tensor_tensor(out=ks2[:], in0=kxr, in1=cosq[:, :32], op=AL.mult)
        nc.vector.tensor_tensor(out=kpe[:, 0:64:2], in0=ks2[:], in1=ks1[:],
                                op=AL.subtract)
        nc.vector.tensor_tensor(out=ks1[:], in0=kxr, in1=sinq[:, :32], op=AL.mult)
        nc.vector.tensor_tensor(out=ks2[:], in0=kxi, in1=cosq[:, :32], op=AL.mult)
        nc.vector.tensor_tensor(out=kpe[:, 1:64:2], in0=ks1[:], in1=ks2[:],
                                op=AL.add)

        # new-token tiles: kv_n bf16 row, kv_nT cols, kpeT col
        kvn_bf = cp.tile([1, KVLR], BF)
        nc.scalar.copy(out=kvn_bf[:], in_=kvn_f[:])
        pt_kv = ppt.tile([128, 64], F32, tag="tr")
        for cc in range(4):
            trans_row(kvn_f[:, cc * 128:(cc + 1) * 128], 128,
                      pt_kv[:, cc:cc + 1])
        trans_row(kpe[:], 64, pt_kv[:64, 4:5])
        kvnT = cp.tile([128, 4], BF)
        nc.scalar.copy(out=kvnT[:], in_=pt_kv[:, :4])
        kpeT = cp.tile([64, 1], BF)
        nc.scalar.copy(out=kpeT[:], in_=pt_kv[:64, 4:5])

        # ================= attention =================
        den = cp.tile([H, 16], F32)
        po = ppa.tile([H, 512], F32)
        kvT_d = I["kvT"].ap()
        kvn_d = I["kvn"].ap()
        peT_d = I["peT"].ap()
        n_mm2 = NTB * 8
        mm2_i = 0
        for tb in range(NTB):
            t0 = tb * TBW
            w = TBW if tb < NTB - 1 else TBW - 1  # prefix cols available
            kvTt = kvTp.tile([128, 4, TBW], BF, tag="kvT")
            nc.sync.dma_start(
                out=kvTt[:, :, :w],
                in_=kvT_d[:, t0:t0 + w].rearrange("(n p) t -> p n t", p=128),
            )
            pet = pep.tile([64, TBW], BF, tag="pe")
            nc.sync.dma_start(out=pet[:, :w], in_=peT_d[:, t0:t0 + w])
            kvnt = kvnp.tile([128, 8, 512], BF, tag="kvn")
            if tb < NTB - 1:
                nc.sync.dma_start(
                    out=kvnt[:],
                    in_=kvn_d[t0:t0 + TBW, :].rearrange("(n p) m -> p n m",
                                                        p=128),
                )
            else:
                nc.sync.dma_start(
                    out=kvnt[:, :7, :],
                    in_=kvn_d[t0:t0 + 896, :].rearrange("(n p) m -> p n m",
                                                        p=128),
                )
                nc.sync.dma_start(
                    out=kvnt[:127, 7, :],
                    in_=kvn_d[t0 + 896:t0 + 1023, :],
                )
                # inject the new token (t = 8191)
                for cc in range(4):
                    nc.vector.tensor_copy(out=kvTt[:, cc, TBW - 1:TBW],
                                          in_=kvnT[:, cc:cc + 1])
                nc.vector.tensor_copy(out=pet[:, TBW - 1:TBW], in_=kpeT[:])
                nc.sync.dma_start(out=kvnt[127:128, 7, :],
                                  in_=kvn_bf[0:1, :])

            for s in range(2):
                ps = pps.tile([H, 512], F32, tag="scores")
                for cc in range(4):
                    nc.tensor.matmul(
                        ps[:], qT[cc], kvTt[:, cc, s * 512:(s + 1) * 512],
                        start=(cc == 0), stop=False,
                    )
                nc.tensor.matmul(ps[:], qpT[:], pet[:, s * 512:(s + 1) * 512],
                                 start=False, stop=True)
                ex = atp.tile([H, 512], BF, tag="exp")
                nc.scalar.activation(out=ex[:], in_=ps[:], func=AF.Exp,
                                     scale=SCALE,
                                     accum_out=den[:, tb * 2 + s:tb * 2 + s + 1])
                ptr = ppt.tile([128, 64], BF, tag="trb")
                for u in range(4):
                    nc.tensor.transpose(ptr[:, u * 16:(u + 1) * 16],
                                        ex[:, u * 128:(u + 1) * 128],
                                        id_bf[0:H, 0:H])
                exT = atp.tile([128, 64], BF, tag="expT")
                nc.scalar.copy(out=exT[:], in_=ptr[:])
                for u in range(4):
                    nc.tensor.matmul(
                        po[:], exT[:, u * 16:(u + 1) * 16],
                        kvnt[:, s * 4 + u, :],
                        start=(mm2_i == 0), stop=(mm2_i == n_mm2 - 1),
                        skip_group_check=True,
                    )
                    mm2_i += 1

        # softmax denominator + normalize
        den1 = cp.tile([H, 1], F32)
        nc.vector.tensor_reduce(out=den1[:], in_=den[:],
                                axis=mybir.AxisListType.X, op=AL.add)
        dinv = cp.tile([H, 1], F32)
        nc.vector.reciprocal(out=dinv[:], in_=den1[:])
        oln = cp.tile([H, 512], BF)
        nc.vector.tensor_scalar(out=oln[:], in0=po[:], scalar1=dinv[:],
                                scalar2=None, op0=AL.mult)

        # transpose o_lat -> [128, 4*16] bf16 (col = cc*16+h)
        pt_o = ppt.tile([128, 64], BF, tag="trb")
        for cc in range(4):
            nc.tensor.transpose(pt_o[:, cc * 16:(cc + 1) * 16],
                                oln[:, cc * 128:(cc + 1) * 128],
                                id_bf[0:H, 0:H])
        olT = cp.tile([128, 64], BF)
        nc.scalar.copy(out=olT[:], in_=pt_o[:])

        # V projection -> oT [128d, 16h] bf16
        wbv_sb = wc.tile([128, 4, H * DV], BF)
        nc.sync.dma_start(out=wbv_sb[:], in_=_chunked(I["wbv"].ap()))
        ps_vo = ppt.tile([128, 64], F32, tag="tr")
        for h in range(H):
            for cc in range(4):
                nc.tensor.matmul(
                    ps_vo[:, h:h + 1],
                    wbv_sb[:, cc, h * 128:(h + 1) * 128],
                    olT[:, cc * 16 + h:cc * 16 + h + 1],
                    start=(cc == 0), stop=(cc == 3),
                )
        oT = cp.tile([128, H], BF)
        nc.scalar.copy(out=oT[:], in_=ps_vo[:, :H])

        # wo projection -> out [1, 2048] f32
        out_sb = cp.tile([1, DIM], F32)
        woT_d = I["woT"].ap()
        for mb in range(4):
            wt = wp.tile([128, 16, 512], BF, tag="wstream")
            nc.sync.dma_start(
                out=wt[:],
                in_=_chunked(woT_d)[:, :, mb * 512:(mb + 1) * 512],
            )
            ps = ppg.tile([1, 512], F32, tag="stage")
            for h in range(H):
                nc.tensor.matmul(ps[:], oT[:, h:h + 1], wt[:, h, :],
                                 start=(h == 0), stop=(h == H - 1))
            nc.vector.tensor_tensor(
                out=out_sb[:, mb * 512:(mb + 1) * 512], in0=ps[:],
                in1=wob[:, mb * 512:(mb + 1) * 512], op=AL.add,
            )
        nc.sync.dma_start(out=out_d.ap(), in_=out_sb[:])


def _prep_inputs(inputs):
    f = {k: np.asarray(v) for k, v in inputs.items()}
    x = f["x"].astype(np.float32).reshape(B, DIM)
    kvp = f["kv_cache_prefix"].astype(np.float32)
    pep_ = f["pe_cache_prefix"].astype(np.float32)
    cos = f["freqs_cos"].astype(np.float32).reshape(-1)[:32]
    sin = f["freqs_sin"].astype(np.float32).reshape(-1)[:32]

    wq_a = f["wq_a_w"].astype(np.float32)
    wq_b = f["wq_b_w"].astype(np.float32)
    wkv_a = f["wkv_a_w"].astype(np.float32)
    wkv_b = f["wkv_b_w"].astype(np.float32).reshape(H, DN + DV, KVLR)
    wo = f["wo_w"].astype(np.float32)

    shared = {
        "wqaT": np.ascontiguousarray(wq_a.T).astype(npbf),
        "wqbT": np.ascontiguousarray(wq_b.T).astype(npbf),
        "wkvaT": np.ascontiguousarray(wkv_a.T).astype(npbf),
        "wbk": np.ascontiguousarray(wkv_b[:, :DN].reshape(H * DN, KVLR)).astype(npbf),
        "wbv": np.ascontiguousarray(
            np.transpose(wkv_b[:, DN:].reshape(H, DV, 4, 128), (2, 3, 0, 1))
            .reshape(512, H * DV)).astype(npbf),
        "woT": np.ascontiguousarray(wo.T).astype(npbf),
        "qnw": f["q_norm_w"].astype(np.float32).reshape(1, QLR),
        "kvnw": f["kv_norm_w"].astype(np.float32).reshape(1, KVLR),
        "wqab": f["wq_a_b"].astype(np.float32).reshape(1, QLR),
        "wqbb": f["wq_b_b"].astype(np.float32).reshape(1, H * (DN + DR)),
        "wkvab": f["wkv_a_b"].astype(np.float32).reshape(1, KVLR + DR),
        "wob": f["wo_b"].astype(np.float32).reshape(1, DIM),
        "cosq": np.tile(cos, H).reshape(1, H * 32),
        "sinq": np.tile(sin, H).reshape(1, H * 32),
    }
    in_maps = []
    for b in range(B):
        m = dict(shared)
        m["xT16"] = np.ascontiguousarray(x[b].reshape(16, 128).T).astype(npbf)
        m["kvT"] = np.ascontiguousarray(kvp[b].T).astype(npbf)
        m["kvn"] = np.ascontiguousarray(kvp[b]).astype(npbf)
        m["peT"] = np.ascontiguousarray(pep_[b].T).astype(npbf)
        in_maps.append(m)
    return in_maps


def run(inputs, trace=False, tmpdir=None):
    nc = _build()
    in_maps = _prep_inputs(inputs)
    res = bass_utils.run_bass_kernel_spmd(
        nc, in_maps, core_ids=list(range(N_CORES)), trace=trace, tmpdir=tmpdir,
    )
    out = np.zeros((B, S, DIM), dtype=np.float32)
    for b in range(B):
        out[b, 0, :] = res.results[b]["out"][0]
    return out, res


def kernel(**inputs) -> np.ndarray:
    out, _ = run(inputs, trace=False)
    return out

